# revision 34
# baseline (speedup 1.0000x reference)
"""Additive attention (Bahdanau) fused Trainium2 kernel, data-parallel over batch.

Math: with q = Q @ Wq.T + bq, k = K @ Wk.T + bk,
  scores[b,i,j] = tanh( w_s . (q[b,i] + k[b,j]) + b_s )
                = tanh( qs[b,i] + ks[b,j] + cbs )
where qs = Q @ u, ks = K @ v, u = Wq.T @ w_s, v = Wk.T @ w_s,
cbs = (bq+bk).w_s + b_s. The (B,Lq,Lk,H) intermediate is never materialized.
u, v, cbs are tiny weight-only folds (O(H*F)) done host-side and shipped as
extra columns of the packed tensors; all O(L*F) math runs on device.

Softmax weights: exp(tanh(x)) is itself a bounded sigmoid-shaped function;
we use the minimax fit  exp(tanh(x)) ~= SIGB * (sigmoid(SIGC*x + SIGD) + AOB)
(max rel err 3.1e-3 on |x|<=5, saturates to the right asymptotes outside).
The global factor SIGB cancels in the softmax, so ONE ACT pass per key chunk
(Sigmoid; the affine pre-map rides the instruction's free scale/bias) plus a
2x-rate bf16 DVE add of AOB replaces a tanh+exp pair.  Masking is host-side
zeroing of the [V | 1] rows for keys j >= valid_len (exact: removes them from
numerator AND denominator, matching the reference's -1e6 fill). The softmax
denominator comes from a ones-column appended to V inside the attn @ V matmul.

All Q and K traffic is fp8 (e4m3) with power-of-2 scales; per-element
quantization noise is iid across keys/features and washes out in the softmax
averaging (measured, not just assumed). cbs rides as a two-fp8-column
hi/lo split (c_hi + c_lo/64), reconstructed in one tiny DVE pass, so the
weight-fold precision stays ~1e-3 while every early-stream byte is fp8.

Layout per core (batch element b), all feature-major so projections run on PE:
  qp0/qp1 [128f, j | u_h]   = Q.T half h + its u column (fp8); two DMAs so
                              qs half 0 starts on qp0's earlier semaphore
  kp8 [128f, c*128+j | v hi/lo | c_hi c_lo | K.T c2-3] = K.T (fp8, *KSCALE)
  vp0/vp1 [128k, cc*257+g]  = [V|1] rows (chunks 0-1 / 2-3), masked rows 0
scores stay TRANSPOSED [k, q]: eT_c = Sigmoid(SIGC/QSCALE * qs_bc + ksb_c)
on ACT, eS_c = eT_c + AOB on DVE (bf16 2x), then eS.T @ [V|1] on the PE
trails each chunk; per-qc reciprocal+scale, two column-sliced stores (the
second issued from the scalar queue's HWDGE ring -- qActDynamicHW -- so the
descriptor-gens overlap and neither store queues behind the input ring).

Timing notes (measured): a DMA's completion semaphore fires ~1.9us after its
descriptor-gen ends (HBM read receipt under load); the 8 cores' input
streams share the DMA/HBM fabric, so arrival order is the scarce resource.
The PE spends the DMA window on warm-up matmuls to open the HAM clock gate
(cold PE = 1.2 GHz; needs sustained matmul activity to reach 2.4), and
N_FILL filler matmuls bridge the projections->chain idle gap so the gate
does not re-close before the chain's accumulation matmuls. qs/ks projection
matmuls are interleaved in arrival order: qs half 0, ks 0-1, qs half 1,
ks 2-3.

Sharding: batch B=8 across 8 NeuronCores, one batch element per core.
"""

from contextlib import ExitStack

import numpy as np
import ml_dtypes

import concourse.tile as tile
from concourse import bacc, mybir
from concourse.bass import ts
from concourse.bass_utils import run_bass_kernel_spmd

B, LQ, LK = 8, 512, 512
F = 256          # feature dim of Q/K/V
H = 128          # hidden dim of the additive-attention MLP
P = 128          # SBUF partitions
QT = LQ // P     # query chunks per core
KT = LK // P     # key chunks per core
FH = F // P      # feature halves
NCORES = 8
N_WARM = 4       # PE warm-up matmuls (512 cols each) to open the HAM gate
N_FILL = 4       # filler matmuls (256 cols) bridging projections -> chain

F32 = mybir.dt.float32
BF16 = mybir.dt.bfloat16
BF = ml_dtypes.bfloat16
FP8 = ml_dtypes.float8_e4m3
F8 = mybir.dt.float8e4
QSCALE = 64.0    # u columns shipped as u*QSCALE in fp8; ACT scale rescales
KSCALE = 32.0    # K shipped as K*KSCALE in fp8
VSCALE = 1024.0  # v columns shipped as v*VSCALE in fp8
CS = 64.0        # cbs hi/lo split scale (c_hi + c_lo/64, both *CS)

# exp(tanh(x)) ~= SIGB*(sigmoid(SIGC*x+SIGD) + AOB); SIGB cancels in softmax
SIGA = 0.3690355303146853
SIGB = 2.3407045472544117
SIGC = 2.142469687764282
SIGD = -0.9968575347084756
AOB = SIGA / SIGB            # 0.15766 additive shift on the sigmoid output
KSINV = SIGC / (KSCALE * VSCALE)

QP_W = LQ + 1                # Q.T half + its u column
V0 = 2 * F                   # kp8a: 512..515: v0_hi v0_lo v1_hi v1_lo
CHI = V0 + 2 * FH            # 516: cbs hi, 517: cbs lo
KPA_W = CHI + 2
KPB_W = 2 * F
VP_W = 2 * (F + 1)           # two [V|1] chunks per vp tensor

TRACE = False
LAST_RESULT = None


def _emit(tc, d):
    nc = tc.nc
    X = mybir.AxisListType
    A = mybir.AluOpType
    AF = mybir.ActivationFunctionType

    with ExitStack() as ctx:
        consts = ctx.enter_context(tc.tile_pool(name="consts", bufs=1))
        big = ctx.enter_context(tc.tile_pool(name="big", bufs=1))
        es_pool = ctx.enter_context(tc.tile_pool(name="es", bufs=4))
        et_pool = ctx.enter_context(tc.tile_pool(name="et", bufs=4))
        ps_fill = ctx.enter_context(tc.tile_pool(name="ps_fill", bufs=1, space="PSUM"))
        ps_qs = ctx.enter_context(tc.tile_pool(name="ps_qs", bufs=1, space="PSUM"))
        ps_ks = ctx.enter_context(tc.tile_pool(name="ps_ks", bufs=1, space="PSUM"))
        ps_acc = ctx.enter_context(tc.tile_pool(name="ps_acc", bufs=1, space="PSUM"))

        # ---- DMA issue: inputs split across BOTH HWDGE rings (sync and
        # scalar) so the two critical tensors drain in parallel and get a
        # double round-robin share of the SDMA engines under fabric
        # contention; FIFO within each ring in need order.
        qp0 = big.tile([P, QP_W], F8)
        nc.sync.dma_start(qp0, d["qp0"])
        qp1 = big.tile([P, QP_W], F8)
        nc.sync.dma_start(qp1, d["qp1"])
        kp8 = big.tile([P, KPA_W + KPB_W], F8)
        nc.scalar.dma_start(kp8, d["kp8"])
        kp8a = kp8[:, 0:KPA_W]
        kp8b = kp8[:, KPA_W:KPA_W + KPB_W]
        vp0 = big.tile([P, VP_W], BF16)
        nc.sync.dma_start(vp0, d["vp0"])
        vp1 = big.tile([P, VP_W], BF16)
        nc.scalar.dma_start(vp1, d["vp1"])

        # PE warm-up operands: a tiny tile memset on the (otherwise idle)
        # gpsimd queue so the FIRST warm matmul issues as early as possible,
        # then a full-width tile from the vector queue for the big warms.
        junk2 = consts.tile([P, 64], BF16)
        nc.gpsimd.memset(junk2, 0.0)
        junk = consts.tile([P, LQ], BF16)
        nc.vector.memset(junk, 0.0)

        # ACT table prefetch: dummy Sigmoid loads sigmoid_and_others (also
        # holds copy) while the DMAs stream.
        warm = consts.tile([1, 1], F32)
        nc.vector.memset(warm, 0.0)
        nc.scalar.activation(warm, warm, AF.Sigmoid)

        # ---- PE warm-up while the input DMAs land: the HAM clock gate
        # needs sustained matmul activity to open; start tiny and early,
        # then switch to full-width volume.
        qs_bc = ps_qs.tile([P, LQ], F32)  # warm-up target; later the qs row
        for _ in range(6):
            nc.tensor.matmul(qs_bc[0:64, 0:64], junk2, junk2, start=True, stop=True)
        for _ in range(N_WARM):
            nc.tensor.matmul(qs_bc, junk[:, 0:P], junk, start=True, stop=True)

        # ---- projections, interleaved in DMA-arrival order ----
        # qs broadcast row: qs_bc[p, j] = sum_f u[f] Q.T[f, j] for all p
        qps = (qp0, qp1)
        nc.tensor.matmul(qs_bc, qp0[:, LQ:LQ + 1].broadcast_to([P, P]),
                         qp0[:, 0:LQ], start=True, stop=False)

        # ks columns, hi/lo: each matmul's moving operand is the adjacent
        # [v_hi | v_lo] fp8 column pair, so chunk cc lands as two psum
        # columns [ks_hi, ks_lo]; ks = ks_hi + ks_lo/64 combined on DVE.
        ks01 = ps_ks.tile([P, 4], F32, name="ks01")
        ks23 = ps_ks.tile([P, 4], F32, name="ks23")

        def ks_mms(dst, kt):
            for cc in range(2):
                for h in range(FH):
                    nc.tensor.matmul(dst[:, 2 * cc:2 * cc + 2],
                                     kt[:, cc * 2 * P + h * P:cc * 2 * P + (h + 1) * P],
                                     kp8a[:, V0 + 2 * h:V0 + 2 * h + 2],
                                     start=(h == 0), stop=(h == 1))

        ks_mms(ks01, kp8a)
        nc.tensor.matmul(qs_bc, qp1[:, LQ:LQ + 1].broadcast_to([P, P]),
                         qp1[:, 0:LQ], start=False, stop=True)
        ks_mms(ks23, kp8b)

        # filler matmuls: keep the PE busy through the sigma0 latency window
        # so the HAM clock gate (which needs SUSTAINED activity) stays on a
        # path to full speed before the chain matmuls arrive.
        fill = ps_fill.tile([P, 2 * P], F32)
        for _ in range(N_FILL):
            nc.tensor.matmul(fill, junk[:, 0:P], junk[:, 0:2 * P],
                             start=True, stop=True)

        # cbs reconstruction: cbs3f = SIGC*cbs + SIGD from the hi/lo fp8 pair
        cbs_hi = consts.tile([P, 1], F32)
        nc.vector.tensor_scalar(cbs_hi, kp8a[:, CHI:CHI + 1],
                                SIGC / CS, SIGD, A.mult, A.add)
        cbs3f = consts.tile([P, 1], F32)
        nc.vector.tensor_scalar(cbs3f, kp8a[:, CHI + 1:CHI + 2],
                                SIGC / (CS * 64.0), cbs_hi, A.mult, A.add)

        # sigmoid bias columns: ksb_c = SIGC*(ks_hi + ks_lo/64 + cbs) + SIGD.
        # One [P,1] tile per chunk so sigma_c waits only its own 2-deep DVE
        # chain (a shared [P,2] tile would make sigma0 wait both columns).
        kscc = [consts.tile([P, 1], F32, name=f"kscc{c}") for c in range(KT)]
        ksbc = [consts.tile([P, 1], F32, name=f"ksbc{c}") for c in range(KT)]
        for c in range(KT):
            kst, cc = (ks01, ks23)[c // 2], c % 2
            nc.vector.tensor_scalar(
                kscc[c], kst[:, 2 * cc + 1:2 * cc + 2],
                1.0 / 64.0, kst[:, 2 * cc:2 * cc + 1], A.mult, A.add)
            nc.vector.tensor_scalar(ksbc[c], kscc[c], KSINV, cbs3f, A.mult, A.add)

        # ---- fused score->weight: eT = sigmoid(SIGC*(qs+ks+cbs)+SIGD) on
        # ACT (ONE pass per chunk), then eS = eT + AOB on DVE (bf16 2x rate);
        # eS.T @ [V | 1] on the PE trails each chunk.
        accs = [ps_acc.tile([P, F + 1], F32, tag=f"acc{qc}", name=f"acc{qc}")
                for qc in range(QT)]
        vtiles = [vp0[:, 0:F + 1], vp0[:, F + 1:2 * (F + 1)],
                  vp1[:, 0:F + 1], vp1[:, F + 1:2 * (F + 1)]]
        for c in range(KT):
            eT = et_pool.tile([P, LQ], BF16, tag="eT")
            nc.scalar.activation(eT, qs_bc, AF.Sigmoid, scale=SIGC / QSCALE,
                                 bias=ksbc[c])
            if c == KT - 1:
                # last chunk: shift in two half-tiles so its matmuls start
                # right behind the first half instead of the full shift
                eSa = es_pool.tile([P, 2 * P], BF16, tag="eSa")
                nc.vector.tensor_scalar(eSa, eT[:, 0:2 * P], AOB, None, A.add)
                eSb = es_pool.tile([P, 2 * P], BF16, tag="eSb")
                nc.vector.tensor_scalar(eSb, eT[:, 2 * P:LQ], AOB, None, A.add)
                halves = (eSa, eSb)
                for qc in range(QT):
                    nc.tensor.matmul(accs[qc],
                                     halves[qc // 2][:, (qc % 2) * P:(qc % 2 + 1) * P],
                                     vtiles[c], start=False, stop=True)
            else:
                eS = es_pool.tile([P, LQ], BF16, tag="eS")
                nc.vector.tensor_scalar(eS, eT, AOB, None, A.add)
                for qc in range(QT):
                    nc.tensor.matmul(accs[qc], eS[:, ts(qc, P)], vtiles[c],
                                     start=(c == 0), stop=False)

        # ---- normalize (ACT takes qc 0/1, DVE takes qc 2/3); two
        # column-sliced stores, the second issued from the DVE queue so the
        # descriptor-gens overlap ----
        ob01 = big.tile([P, 2 * F], BF16)
        ob23 = big.tile([P, 2 * F], BF16)
        recs = [consts.tile([P, 1], F32, tag=f"rec{qc}", name=f"rec{qc}")
                for qc in range(QT)]
        for qc in range(QT):
            nc.vector.reciprocal(recs[qc], accs[qc][:, F:F + 1])
        nc.scalar.activation(ob01[:, 0:F], accs[0][:, 0:F], AF.Copy,
                             bias=0.0, scale=recs[0])
        nc.scalar.activation(ob01[:, F:2 * F], accs[1][:, 0:F], AF.Copy,
                             bias=0.0, scale=recs[1])
        nc.sync.dma_start(d["out"][:, 0:2 * F], ob01)
        nc.vector.tensor_scalar(ob23[:, 0:F], accs[2][:, 0:F], recs[2], None, A.mult)
        nc.vector.tensor_scalar(ob23[:, F:2 * F], accs[3][:, 0:F], recs[3], None, A.mult)
        nc.scalar.dma_start(d["out"][:, 2 * F:4 * F], ob23)

        # late read of the warm-up/fill psum banks so those matmuls stay live
        warm_junk = consts.tile([P, 1], F32)
        nc.vector.reduce_sum(warm_junk, qs_bc[:, 0:8], axis=X.X)
        nc.vector.reduce_sum(warm_junk, fill[:, 0:8], axis=X.X)


_NC = None


def _build_nc():
    nc = bacc.Bacc("TRN2", target_bir_lowering=False, debug=False, num_devices=1)
    d = {}
    d["qp0"] = nc.dram_tensor("qp0", [P, QP_W], F8, kind="ExternalInput").ap()
    d["qp1"] = nc.dram_tensor("qp1", [P, QP_W], F8, kind="ExternalInput").ap()
    d["kp8"] = nc.dram_tensor("kp8", [P, KPA_W + KPB_W], F8, kind="ExternalInput").ap()
    d["vp0"] = nc.dram_tensor("vp0", [P, VP_W], BF16, kind="ExternalInput").ap()
    d["vp1"] = nc.dram_tensor("vp1", [P, VP_W], BF16, kind="ExternalInput").ap()
    d["out"] = nc.dram_tensor("out", [P, QT * F], BF16, kind="ExternalOutput").ap()

    with tile.TileContext(nc) as tc:
        _emit(tc, d)
    nc.compile()
    return nc


def get_nc():
    global _NC
    if _NC is None:
        _NC = _build_nc()
    return _NC


def make_in_maps(queries, keys, values, valid_lens, Wq, bq, Wk, bk, w_s, b_s):
    f32 = lambda a: np.asarray(a, dtype=np.float32)
    qs, ks, vs = f32(queries), f32(keys), f32(values)
    vl = np.asarray(valid_lens)
    ws = f32(w_s)
    u = f32(Wq).T @ ws            # [F]
    v = f32(Wk).T @ ws            # [F]
    cbs = float(ws @ (f32(bq) + f32(bk)) + f32(b_s).reshape(-1)[0])
    # two-fp8 hi/lo split of cbs*CS (c_hi + c_lo/64)
    c_hi = np.float32(FP8(np.float32(cbs * CS)))
    c_lo = np.float32(FP8(np.float32((cbs * CS - c_hi) * 64.0)))

    in_maps = []
    for b in range(NCORES):
        # qp_h[p, j] = Q[j, h*128 + p]; last col = u_h * QSCALE
        qT = qs[b].T.reshape(FH, P, LQ)
        uh = QSCALE * u.reshape(FH, P)
        qpk = [np.concatenate([qT[h], uh[h][:, None]], axis=1) for h in range(FH)]
        # k chunk tiles: [p, c*256 + h*128 + j] = K[c*128 + j, h*128 + p]
        kd = KSCALE * ks[b].T.reshape(FH, P, KT, P).transpose(1, 2, 0, 3).reshape(P, KT * F)
        kpk = np.empty((P, KPA_W), np.float32)
        kpk[:, 0:2 * F] = kd[:, 0:2 * F]
        vsc = VSCALE * v.reshape(FH, P).T          # [P, FH]
        v_hi = np.float32(vsc.astype(FP8))
        v_lo = (vsc - v_hi) * 64.0
        kpk[:, V0:V0 + 2 * FH:2] = v_hi
        kpk[:, V0 + 1:V0 + 2 * FH:2] = v_lo
        kpk[:, CHI] = c_hi
        kpk[:, CHI + 1] = c_lo
        # vp[p, cc*257 + g] = V[c*128 + p, g] (g<256) | 1.0 (g=256),
        # rows for masked keys (c*128+p >= valid_len) zeroed
        va = np.ones((KT, P, F + 1), np.float32)
        va[:, :, :F] = vs[b].reshape(KT, P, F)
        kidx = (np.arange(KT)[:, None] * P + np.arange(P)[None, :])
        va[kidx >= int(vl[b])] = 0.0
        vap = va.transpose(1, 0, 2).reshape(P, KT * (F + 1))
        in_maps.append({
            "qp0": qpk[0].astype(FP8),
            "qp1": qpk[1].astype(FP8),
            "kp8": np.concatenate([kpk, kd[:, 2 * F:4 * F]], axis=1).astype(FP8),
            "vp0": np.ascontiguousarray(vap[:, 0:VP_W]).astype(BF),
            "vp1": np.ascontiguousarray(vap[:, VP_W:2 * VP_W]).astype(BF),
        })
    return in_maps


def kernel(queries, keys, values, valid_lens, Wq, bq, Wk, bk, w_s, b_s):
    global LAST_RESULT
    nc = get_nc()
    in_maps = make_in_maps(queries, keys, values, valid_lens, Wq, bq, Wk, bk, w_s, b_s)
    res = run_bass_kernel_spmd(nc, in_maps, list(range(NCORES)), trace=TRACE)
    LAST_RESULT = res
    out = np.stack([np.asarray(res.results[b]["out"]) for b in range(NCORES)], axis=0)
    # [P, QT*F] (bf16) -> [LQ, F] fp32
    out = out.astype(np.float32).reshape(B, P, QT, F).transpose(0, 2, 1, 3).reshape(B, LQ, F)
    return np.ascontiguousarray(out)
